# revision 24
# baseline (speedup 1.0000x reference)
"""DeltaSepConvGRU Trainium2 kernel.

Sharding: 8 cores = batch(4) x width-split(2). Each core gets an 88-column
window (80 output cols + 8-col halo on the interior side) of one batch image
and computes the full two-step GRU locally; no collectives. Horizontal convs
need the halo (4 chained (1,5) convs -> 8 cols); vertical (5,1) convs run on
the full 64-row height. Conv zero-padding is realized by clamping taps to
valid input ranges (skipped taps contribute zero, matching the reference's
zero padding); clamped matmuls write row/column-offset PSUM subranges, with a
full-coverage matmul always first in each accumulation group.

All conv matmuls run in float32r (fp32 with 11-bit mantissa, full PE speed);
elementwise math is fp32. Stage intermediates stream through packed DRAM
scratch tensors (one big contiguous DMA instead of several small ones) and
DMA dispatch is split between the SP and Pool sequencers.
"""
import sys
sys.path.insert(0, '/opt/trn_rl_repo')
import numpy as np

B, HID, INP = 4, 128, 320
H, W = 64, 160
WC = 88           # per-core window width
WP = WC + 4       # padded width for horizontal-conv inputs
RB = 8            # rows per block
NB = H // RB
NG = 2            # psum groups per block
GR = 4            # rows per group
N = GR * WC       # matmul free size (352)
THR = 0.02
KS = [128, 112, 112, 96]    # ci tiles: t0 = h, t1..t3 = x (all round to tile_size 128)
XOFF = [0, 112, 224]        # x-channel offsets of tiles t1..t3
WNAMES = ["z1", "r1", "q1", "z2", "r2", "q2"]
TAPS = [2, 0, 1, 3, 4]      # full-coverage tap first (s=0)

_CACHE = {}
REPS = 1
MODE = "full"  # full | pe | ew | nodma


def _build_nc():
    import concourse.bass as bass
    from concourse import bacc, mybir
    import concourse.tile as tile

    f32 = mybir.dt.float32
    f32r = mybir.dt.float32r
    AF = mybir.ActivationFunctionType
    ALU = mybir.AluOpType

    nc = bacc.Bacc(None, target_bir_lowering=False, debug=False)

    h_d = nc.dram_tensor("h", [128, H, WC], f32r, kind="ExternalInput")
    x1_d = nc.dram_tensor("x1", [INP, H, WC], f32r, kind="ExternalInput")
    x2_d = nc.dram_tensor("x2", [INP, H, WC], f32r, kind="ExternalInput")
    w_d = {g: nc.dram_tensor(f"w_{g}", [128, 4, 5, 128], f32, kind="ExternalInput")
           for g in WNAMES}
    ball_d = nc.dram_tensor("ball", [128, 6], f32, kind="ExternalInput")
    ident_d = nc.dram_tensor("ident", [128, 128], f32, kind="ExternalInput")
    zeros_d = nc.dram_tensor("zeros", [128, 128], f32r, kind="ExternalInput")
    out_d = nc.dram_tensor("out", [128, H, WC], f32, kind="ExternalOutput")

    # Packed DRAM scratch
    S_czrq1 = nc.dram_tensor("S_czrq1", [128, 3, H, WC], f32r)   # cz1,cr1,cq1
    S_rh1 = nc.dram_tensor("S_rh1", [128, 2, H, WC], f32r)       # r1h, hmid1
    S_bz = nc.dram_tensor("S_bz", [128, 4, H, WC], f32r)         # cz2,cr2,xq2,r2h
    S_cqh1 = nc.dram_tensor("S_cqh1", [128, 2, H, WC], f32r)     # cq2, h1
    S_dx = nc.dram_tensor("S_dx", [128, 3, H, WC], f32r)
    S_hm2 = nc.dram_tensor("S_hm2", [128, H, WC], f32r)
    S_de = nc.dram_tensor("S_de", [128, 2, H, WC], f32r)         # dhmid, dr2h

    def dma(out, in_):
        nc.sync.dma_start(out=out, in_=in_)

    def dmap(out, in_):
        nc.gpsimd.dma_start(out=out, in_=in_)

    def st(out, in_, via):
        if MODE == "pe":
            return
        via(out, in_)

    def zin(p, rows):
        return bass.AP(tensor=zeros_d[:].tensor, offset=0,
                       ap=[[128, p], [0, rows], [1, 2]])

    def zero_borders(tile4, rb, nt=None):
        """Zero the 2-col borders of a WP-padded tile; pool slots keep the
        zeros across reuse, so only the first `bufs` blocks need it."""
        if rb >= 3:
            return
        if nt is None:
            dmap(tile4[:, :, 0:2], zin(128, tile4.shape[1]))
            dmap(tile4[:, :, 90:92], zin(128, tile4.shape[1]))
        else:
            for t in range(nt):
                dmap(tile4[:, t, :, 0:2], zin(128, tile4.shape[2]))
                dmap(tile4[:, t, :, 90:92], zin(128, tile4.shape[2]))

    def slot_ap(src, nslots, slot0, step, count, r0, rows):
        """AP over packed scratch [128, nslots, H, WC]: slots
        slot0, slot0+step, ...; rows [r0, r0+rows)."""
        hw_ = H * WC
        return bass.AP(tensor=src[:].tensor, offset=slot0 * hw_ + r0 * WC,
                       ap=[[nslots * hw_, 128], [step * hw_, count],
                           [WC, rows], [1, WC]])

    def x_load(tile_, src, r0, rows, via=None, per_slot=False):
        """Load x-like [320,H,WC] rows into packed [128,3,rows,WC] tile view."""
        d = via or dma
        hw_ = H * WC
        if per_slot:
            for t in range(3):
                d(tile_[0:KS[t + 1], t, :, :],
                  bass.AP(tensor=src[:].tensor,
                          offset=XOFF[t] * hw_ + r0 * WC,
                          ap=[[hw_, KS[t + 1]], [WC, rows], [1, WC]]))
            return
        d(tile_[0:112, 0:2, :, :],
          bass.AP(tensor=src[:].tensor, offset=r0 * WC,
                  ap=[[hw_, 112], [112 * hw_, 2], [WC, rows], [1, WC]]))
        d(tile_[0:96, 2, :, :],
          bass.AP(tensor=src[:].tensor, offset=224 * hw_ + r0 * WC,
                  ap=[[hw_, 96], [WC, rows], [1, WC]]))

    def dx_load(tile_, r0, rows, via=None, per_slot=False):
        d = via or dma
        hw_ = H * WC
        if per_slot:
            for t in range(3):
                d(tile_[0:KS[t + 1], t, :, :],
                  bass.AP(tensor=S_dx[:].tensor, offset=t * hw_ + r0 * WC,
                          ap=[[3 * hw_, KS[t + 1]], [WC, rows], [1, WC]]))
            return
        d(tile_[0:112, 0:2, :, :],
          bass.AP(tensor=S_dx[:].tensor, offset=r0 * WC,
                  ap=[[3 * hw_, 112], [hw_, 2], [WC, rows], [1, WC]]))
        d(tile_[0:96, 2, :, :],
          bass.AP(tensor=S_dx[:].tensor, offset=2 * hw_ + r0 * WC,
                  ap=[[3 * hw_, 96], [WC, rows], [1, WC]]))

    def mm_run(psum3, mms):
        if MODE == "ew":
            return
        for i, (lh, rh, orows, ocols) in enumerate(mms):
            nc.tensor.matmul(psum3[:, orows, ocols], lhsT=lh, rhs=rh,
                             start=(i == 0), stop=(i == len(mms) - 1))

    def mm_horiz(psum3, w, h3, x4, g, extra=None):
        # h3/x4 are WP-padded tiles with zeroed 2-col borders.
        rows = slice(g * GR, (g + 1) * GR)
        mms = []
        if extra is not None:
            mms.append((extra[0], extra[1], slice(0, GR), slice(0, WC)))
        for k in TAPS:
            mms.append((w[:, 0, k, :], h3[:, rows, k:k + WC],
                        slice(0, GR), slice(0, WC)))
            for t in (1, 2, 3):
                mms.append((w[0:KS[t], t, k, :],
                            x4[0:KS[t], t - 1, rows, k:k + WC],
                            slice(0, GR), slice(0, WC)))
        mm_run(psum3, mms)

    def mm_vert(psum3, w, vh3, vx4, g, rb, extra=None, x_only=False,
                h_only=False):
        r0 = rb * RB
        q0 = r0 + g * GR
        base = r0 - 2      # global row of local row 0 in the 12-row tile
        mms = []
        if extra is not None:
            mms.append((extra[0], extra[1], slice(0, GR), slice(0, WC)))
        for k in TAPS:
            s = k - 2
            a_r = max(q0, -s)
            b_r = min(q0 + GR, H - s)
            orows = slice(a_r - q0, b_r - q0)
            la, lb = a_r + s - base, b_r + s - base
            if not x_only:
                mms.append((w[:, 0, k, :], vh3[:, la:lb, :], orows,
                            slice(0, WC)))
            if not h_only:
                for t in (1, 2, 3):
                    mms.append((w[0:KS[t], t, k, :],
                                vx4[0:KS[t], t - 1, la:lb, :], orows,
                                slice(0, WC)))
        mm_run(psum3, mms)

    def p3(psum):
        return psum[:].rearrange("p (r w) -> p r w", w=WC)

    def p3c(psum):
        return psum[:].rearrange("p (w r) -> p w r", w=WC)

    def crv(ap):
        return ap.rearrange("p r c -> p c r")

    def act(out, in_, func, bias=0.0):
        if MODE == "pe":
            return
        nc.scalar.activation(out=out, in_=in_, func=func, bias=bias, scale=1.0)

    def tt(out, in0, in1, op):
        if MODE == "pe":
            return
        nc.vector.tensor_tensor(out=out, in0=in0, in1=in1, op=op)

    def delta_chain(t, sq, d, new, old):
        if MODE == "pe":
            return
        tt(t, new, old, ALU.subtract)
        tt(sq, t, t, ALU.mult)
        nc.vector.scalar_tensor_tensor(out=d, in0=sq, scalar=THR * THR,
                                       in1=t, op0=ALU.is_gt, op1=ALU.mult)

    def halo_rows(rb):
        """(local row range loaded, global row range) of a 12-row halo tile."""
        r0 = rb * RB
        lo, hi = max(r0 - 2, 0), min(r0 + RB + 2, H)
        return lo - (r0 - 2), hi - (r0 - 2), lo, hi

    with tile.TileContext(nc) as tc:
        with tc.tile_pool(name="glob", bufs=1) as gp:
            ident_stg = gp.tile([128, 128], f32)
            dma(ident_stg[:], ident_d[:])
            ident_r = gp.tile([128, 128], f32r)
            nc.vector.tensor_copy(out=ident_r[:], in_=ident_stg[:])
            ball = gp.tile([128, 6], f32)
            dma(ball[:], ball_d[:])
            bias = {g: ball[:, i:i + 1] for i, g in enumerate(WNAMES)}

            wsall = {}
            with tc.tile_pool(name="wstgp", bufs=2) as wsp:
                for g in WNAMES:
                    stg = wsp.tile([128, 4, 5, 128], f32, tag="wstg",
                                   name=f"wstg_{g}")
                    dma(stg[:], w_d[g][:])
                    wsall[g] = gp.tile([128, 4, 5, 128], f32r, tag=f"w_{g}",
                                       name=f"w_{g}")
                    nc.vector.tensor_copy(out=wsall[g][:], in_=stg[:])

            for _rep in range(REPS):
              # ------------- Stage X: d_x = delta(x2 - x1) -> S_dx ----------
              with tc.tile_pool(name="stX", bufs=2) as sp:
                for rb in range(NB):
                    r0 = rb * RB
                    x1X = sp.tile([128, 3, RB, WC], f32r, tag="x1X", bufs=3)
                    x2X = sp.tile([128, 3, RB, WC], f32r, tag="x2X", bufs=3)
                    x_load(x1X, x1_d, r0, RB)
                    x_load(x2X, x2_d, r0, RB)
                    tX = sp.tile([128, 3, RB, WC], f32, tag="tX")
                    sqX = sp.tile([128, 3, RB, WC], f32, tag="sqX")
                    dxX = sp.tile([128, 3, RB, WC], f32r, tag="dxX")
                    for pr, ts in ((slice(0, 112), slice(0, 2)),
                                   (slice(0, 96), slice(2, 3))):
                        delta_chain(tX[pr, ts], sqX[pr, ts], dxX[pr, ts],
                                    x2X[pr, ts].bitcast(f32),
                                    x1X[pr, ts].bitcast(f32))
                    st(S_dx[0:112, 0:2, r0:r0 + RB, :], dxX[0:112, 0:2], dmap)
                    st(S_dx[0:96, 2, r0:r0 + RB, :], dxX[0:96, 2], dmap)

              # ------------- Stage A: step1 horizontal ----------------------
              with tc.tile_pool(name="stA", bufs=2) as sp, \
                   tc.tile_pool(name="psA", bufs=2, space="PSUM") as pp:
                ws = wsall
                for rb in range(NB):
                    r0 = rb * RB
                    hA = sp.tile([128, RB, WP], f32r, tag="hA", bufs=3)
                    zero_borders(hA, rb)
                    dma(hA[:, :, 2:90], h_d[:, r0:r0 + RB, :])
                    xA = sp.tile([128, 3, RB, WP], f32r, tag="xA", bufs=3)
                    zero_borders(xA, rb, nt=3)
                    x_load(xA[:, :, :, 2:90], x1_d, r0, RB, per_slot=True)
                    czrqA = sp.tile([128, 3, RB, WC], f32r, tag="czrqA")
                    r1hA = sp.tile([128, RB, WP], f32r, tag="r1hA", bufs=3)
                    zero_borders(r1hA, rb)
                    hmA = sp.tile([128, RB, WC], f32r, tag="hmA")
                    zA = sp.tile([128, RB, WC], f32, tag="zA")
                    rA = sp.tile([128, RB, WC], f32, tag="rA")
                    qA = sp.tile([128, RB, WC], f32, tag="qA")
                    for g in range(NG):
                        rows = slice(g * GR, (g + 1) * GR)
                        pz = pp.tile([128, N], f32, tag="pz")
                        mm_horiz(p3(pz), ws["z1"], hA, xA, g)
                        act(czrqA[:, 0, rows, :], p3(pz), AF.Identity,
                            bias["z1"])
                        act(zA[:, rows, :], p3(pz), AF.Sigmoid, bias["z1"])
                        pr = pp.tile([128, N], f32, tag="pr")
                        mm_horiz(p3(pr), ws["r1"], hA, xA, g)
                        act(czrqA[:, 1, rows, :], p3(pr), AF.Identity,
                            bias["r1"])
                        act(rA[:, rows, :], p3(pr), AF.Sigmoid, bias["r1"])
                        tt(r1hA[:, rows, 2:90], rA[:, rows, :],
                           hA[:, rows, 2:90].bitcast(f32), ALU.mult)
                        pq = pp.tile([128, N], f32, tag="pq")
                        mm_horiz(p3(pq), ws["q1"], r1hA, xA, g)
                        act(czrqA[:, 2, rows, :], p3(pq), AF.Identity,
                            bias["q1"])
                        act(qA[:, rows, :], p3(pq), AF.Tanh, bias["q1"])
                    t1 = sp.tile([128, RB, WC], f32, tag="t1A")
                    tt(t1[:], qA[:], hA[:, :, 2:90].bitcast(f32), ALU.subtract)
                    t2 = sp.tile([128, RB, WC], f32, tag="t2A")
                    tt(t2[:], zA[:], t1[:], ALU.mult)
                    tt(hmA[:], hA[:, :, 2:90].bitcast(f32), t2[:], ALU.add)
                    st(S_czrq1[:, :, r0:r0 + RB, :], czrqA[:], dmap)
                    st(S_rh1[:, 0, r0:r0 + RB, :], r1hA[:, :, 2:90], dmap)
                    st(S_rh1[:, 1, r0:r0 + RB, :], hmA[:], dmap)

              # ------------- Stage B: step1 vertical (cz2,cr2,xq2,r2h) ------
              with tc.tile_pool(name="stB", bufs=2) as sp, \
                   tc.tile_pool(name="psB", bufs=2, space="PSUM") as pp:
                ws = wsall
                for rb in range(NB):
                    r0 = rb * RB
                    la, lb, glo, ghi = halo_rows(rb)
                    vh = sp.tile([128, RB + 4, WC], f32r, tag="vhB", bufs=3)
                    dma(vh[:, la:lb, :], slot_ap(S_rh1, 2, 1, 1, 1, glo,
                                                 ghi - glo))
                    vx = sp.tile([128, 3, RB + 4, WC], f32r, tag="vxB", bufs=3)
                    x_load(vx[:, :, la:lb, :], x1_d, glo, ghi - glo)
                    bzB = sp.tile([128, 4, RB, WC], f32r, tag="bzB")
                    rB = sp.tile([128, RB, WC], f32, tag="rB")
                    for g in range(NG):
                        rows = slice(g * GR, (g + 1) * GR)
                        pz = pp.tile([128, N], f32, tag="pzB")
                        mm_vert(p3(pz), ws["z2"], vh, vx, g, rb)
                        act(bzB[:, 0, rows, :], p3(pz), AF.Identity,
                            bias["z2"])
                        pr = pp.tile([128, N], f32, tag="prB")
                        mm_vert(p3(pr), ws["r2"], vh, vx, g, rb)
                        act(bzB[:, 1, rows, :], p3(pr), AF.Identity,
                            bias["r2"])
                        act(rB[:, rows, :], p3(pr), AF.Sigmoid, bias["r2"])
                        px = pp.tile([128, N], f32, tag="pxB")
                        mm_vert(p3(px), ws["q2"], vh, vx, g, rb, x_only=True)
                        act(bzB[:, 2, rows, :], p3(px), AF.Identity,
                            bias["q2"])
                    tt(bzB[:, 3], rB[:], vh[:, 2:RB + 2, :].bitcast(f32),
                       ALU.mult)
                    st(S_bz[:, :, r0:r0 + RB, :], bzB[:], dmap)

              # ------------- Stage C: step1 vertical (cq2, h1) --------------
              with tc.tile_pool(name="stC", bufs=2) as sp, \
                   tc.tile_pool(name="psC", bufs=2, space="PSUM") as pp:
                ws = wsall
                for rb in range(NB):
                    r0 = rb * RB
                    la, lb, glo, ghi = halo_rows(rb)
                    vr = sp.tile([128, RB + 4, WC], f32r, tag="vrC", bufs=3)
                    dma(vr[:, la:lb, :], slot_ap(S_bz, 4, 3, 1, 1, glo,
                                                 ghi - glo))
                    czxC = sp.tile([128, 2, RB, WC], f32r, tag="czxC", bufs=3)
                    dma(czxC[:], slot_ap(S_bz, 4, 0, 2, 2, r0, RB))
                    hm1C = sp.tile([128, RB, WC], f32r, tag="hm1C", bufs=3)
                    dma(hm1C[:], slot_ap(S_rh1, 2, 1, 1, 1, r0, RB))
                    chC = sp.tile([128, 2, RB, WC], f32r, tag="chC")
                    qC = sp.tile([128, RB, WC], f32, tag="qC")
                    for g in range(NG):
                        rows = slice(g * GR, (g + 1) * GR)
                        pq = pp.tile([128, N], f32, tag="pqC")
                        mm_vert(p3(pq), ws["q2"], vr, None, g, rb, h_only=True,
                                extra=(ident_r[:], czxC[:, 1, rows, :]))
                        act(chC[:, 0, rows, :], p3(pq), AF.Identity)
                        act(qC[:, rows, :], p3(pq), AF.Tanh)
                    zC = sp.tile([128, RB, WC], f32, tag="zC")
                    act(zC[:], czxC[:, 0].bitcast(f32), AF.Sigmoid)
                    t1 = sp.tile([128, RB, WC], f32, tag="t1C")
                    tt(t1[:], qC[:], hm1C[:].bitcast(f32), ALU.subtract)
                    t2 = sp.tile([128, RB, WC], f32, tag="t2C")
                    tt(t2[:], zC[:], t1[:], ALU.mult)
                    tt(chC[:, 1], hm1C[:].bitcast(f32), t2[:], ALU.add)
                    st(S_cqh1[:, :, r0:r0 + RB, :], chC[:], dmap)

              # ------------- Stage D: step2 horizontal ----------------------
              with tc.tile_pool(name="stD", bufs=2) as sp, \
                   tc.tile_pool(name="psD", bufs=2, space="PSUM") as pp:
                ws = wsall
                for rb in range(NB):
                    r0 = rb * RB
                    hD = sp.tile([128, RB, WC], f32r, tag="hD", bufs=3)
                    dmap(hD[:], h_d[:, r0:r0 + RB, :])
                    h1D = sp.tile([128, RB, WC], f32r, tag="h1D", bufs=3)
                    dmap(h1D[:], slot_ap(S_cqh1, 2, 1, 1, 1, r0, RB))
                    dx = sp.tile([128, 3, RB, WP], f32r, tag="dxD", bufs=3)
                    zero_borders(dx, rb, nt=3)
                    dx_load(dx[:, :, :, 2:90], r0, RB, via=dmap, per_slot=True)
                    r1hD = sp.tile([128, RB, WC], f32r, tag="r1hD", bufs=3)
                    dmap(r1hD[:], slot_ap(S_rh1, 2, 0, 1, 1, r0, RB))
                    czrqD = sp.tile([128, 3, RB, WC], f32r, tag="czrqD", bufs=3)
                    dmap(czrqD[:], S_czrq1[:, :, r0:r0 + RB, :])

                    tD = sp.tile([128, RB, WC], f32, tag="tD")
                    sqD = sp.tile([128, RB, WC], f32, tag="sqD")
                    dh = sp.tile([128, RB, WP], f32r, tag="dh", bufs=3)
                    zero_borders(dh, rb)
                    delta_chain(tD[:], sqD[:], dh[:, :, 2:90],
                                h1D[:].bitcast(f32), hD[:].bitcast(f32))
                    dr1h = sp.tile([128, RB, WP], f32r, tag="dr1h", bufs=3)
                    zero_borders(dr1h, rb)
                    zD = sp.tile([128, RB, WC], f32, tag="zD")
                    qD = sp.tile([128, RB, WC], f32, tag="qD")
                    for g in range(NG):
                        rows = slice(g * GR, (g + 1) * GR)
                        pz = pp.tile([128, N], f32, tag="pzD")
                        mm_horiz(p3(pz), ws["z1"], dh, dx, g,
                                 extra=(ident_r[:], czrqD[:, 0, rows, :]))
                        act(zD[:, rows, :], p3(pz), AF.Sigmoid)
                        prr = pp.tile([128, N], f32, tag="prD")
                        mm_horiz(p3(prr), ws["r1"], dh, dx, g,
                                 extra=(ident_r[:], czrqD[:, 1, rows, :]))
                        rD = sp.tile([128, GR, WC], f32, tag="rD")
                        act(rD[:], p3(prr), AF.Sigmoid)
                        r1n = sp.tile([128, GR, WC], f32, tag="r1n")
                        tt(r1n[:], rD[:], h1D[:, rows, :].bitcast(f32),
                           ALU.mult)
                        tg = sp.tile([128, GR, WC], f32, tag="tg")
                        sqg = sp.tile([128, GR, WC], f32, tag="sqg")
                        delta_chain(tg[:], sqg[:], dr1h[:, rows, 2:90], r1n[:],
                                    r1hD[:, rows, :].bitcast(f32))
                        pq = pp.tile([128, N], f32, tag="pqD")
                        mm_horiz(p3(pq), ws["q1"], dr1h, dx, g,
                                 extra=(ident_r[:], czrqD[:, 2, rows, :]))
                        act(qD[:, rows, :], p3(pq), AF.Tanh)
                    t1 = sp.tile([128, RB, WC], f32, tag="t1D")
                    tt(t1[:], qD[:], h1D[:].bitcast(f32), ALU.subtract)
                    t2 = sp.tile([128, RB, WC], f32, tag="t2D")
                    tt(t2[:], zD[:], t1[:], ALU.mult)
                    hm2D = sp.tile([128, RB, WC], f32r, tag="hm2D")
                    tt(hm2D[:], h1D[:].bitcast(f32), t2[:], ALU.add)
                    st(S_hm2[:, r0:r0 + RB, :], hm2D[:], dma)

              # ------------- Stage E: step2 vertical pass A -----------------
              with tc.tile_pool(name="stE", bufs=2) as sp, \
                   tc.tile_pool(name="psE", bufs=2, space="PSUM") as pp:
                ws = wsall
                for rb in range(NB):
                    r0 = rb * RB
                    la, lb, glo, ghi = halo_rows(rb)
                    hm2E = sp.tile([128, RB + 4, WC], f32r, tag="hm2E", bufs=3)
                    dmap(hm2E[:, la:lb, :], S_hm2[:, glo:ghi, :])
                    hm1E = sp.tile([128, RB + 4, WC], f32r, tag="hm1E", bufs=3)
                    dmap(hm1E[:, la:lb, :], slot_ap(S_rh1, 2, 1, 1, 1, glo,
                                                   ghi - glo))
                    dxE = sp.tile([128, 3, RB + 4, WC], f32r, tag="dxE", bufs=3)
                    dx_load(dxE[:, :, la:lb, :], glo, ghi - glo, via=dmap)
                    crrE = sp.tile([128, 2, RB, WC], f32r, tag="crrE", bufs=3)
                    dmap(crrE[:], slot_ap(S_bz, 4, 1, 2, 2, r0, RB))

                    tE = sp.tile([128, RB + 4, WC], f32, tag="tE")
                    sqE = sp.tile([128, RB + 4, WC], f32, tag="sqE")
                    dhm = sp.tile([128, RB + 4, WC], f32r, tag="dhm")
                    delta_chain(tE[:, la:lb, :], sqE[:, la:lb, :],
                                dhm[:, la:lb, :],
                                hm2E[:, la:lb, :].bitcast(f32),
                                hm1E[:, la:lb, :].bitcast(f32))
                    rE = sp.tile([128, RB, WC], f32, tag="rE")
                    for g in range(NG):
                        rows = slice(g * GR, (g + 1) * GR)
                        pr = pp.tile([128, N], f32, tag="prE")
                        mm_vert(p3(pr), ws["r2"], hm2E if MODE == "pe" else dhm, dxE, g, rb,
                                extra=(ident_r[:], crrE[:, 0, rows, :]))
                        act(rE[:, rows, :], p3(pr), AF.Sigmoid)
                    deE = sp.tile([128, 2, RB, WC], f32r, tag="deE")
                    r2n = sp.tile([128, RB, WC], f32, tag="r2n")
                    tt(r2n[:], rE[:], hm2E[:, 2:RB + 2, :].bitcast(f32),
                       ALU.mult)
                    tE2 = sp.tile([128, RB, WC], f32, tag="tE2")
                    sqE2 = sp.tile([128, RB, WC], f32, tag="sqE2")
                    delta_chain(tE2[:], sqE2[:], deE[:, 1], r2n[:],
                                crrE[:, 1].bitcast(f32))
                    if MODE != "pe":
                        nc.vector.tensor_copy(out=deE[:, 0],
                                              in_=dhm[:, 2:RB + 2, :])
                    st(S_de[:, :, r0:r0 + RB, :], deE[:], dma)

              # ------------- Stage F: step2 vertical pass B -----------------
              with tc.tile_pool(name="stF", bufs=2) as sp, \
                   tc.tile_pool(name="psF", bufs=2, space="PSUM") as pp:
                ws = wsall
                for rb in range(NB):
                    r0 = rb * RB
                    la, lb, glo, ghi = halo_rows(rb)
                    deF = sp.tile([128, 2, RB + 4, WC], f32r, tag="deF", bufs=3)
                    dmap(deF[:, :, la:lb, :],
                        bass.AP(tensor=S_de[:].tensor, offset=glo * WC,
                                ap=[[2 * H * WC, 128], [H * WC, 2],
                                    [WC, ghi - glo], [1, WC]]))
                    dxF = sp.tile([128, 3, RB + 4, WC], f32r, tag="dxF", bufs=3)
                    dx_load(dxF[:, :, la:lb, :], glo, ghi - glo, via=dmap)
                    czF = sp.tile([128, RB, WC], f32r, tag="czF", bufs=3)
                    dmap(czF[:], slot_ap(S_bz, 4, 0, 1, 1, r0, RB))
                    cqF = sp.tile([128, RB, WC], f32r, tag="cqF", bufs=3)
                    dmap(cqF[:], slot_ap(S_cqh1, 2, 0, 1, 1, r0, RB))
                    hm2F = sp.tile([128, RB, WC], f32r, tag="hm2F", bufs=3)
                    dmap(hm2F[:], S_hm2[:, r0:r0 + RB, :])
                    zF = sp.tile([128, RB, WC], f32, tag="zF")
                    qF = sp.tile([128, RB, WC], f32, tag="qF")
                    for g in range(NG):
                        rows = slice(g * GR, (g + 1) * GR)
                        pz = pp.tile([128, N], f32, tag="pzF")
                        mm_vert(p3(pz), ws["z2"], deF[:, 0], dxF, g, rb,
                                extra=(ident_r[:], czF[:, rows, :]))
                        act(zF[:, rows, :], p3(pz), AF.Sigmoid)
                        pq = pp.tile([128, N], f32, tag="pqF")
                        mm_vert(p3(pq), ws["q2"], deF[:, 1], dxF, g, rb,
                                extra=(ident_r[:], cqF[:, rows, :]))
                        act(qF[:, rows, :], p3(pq), AF.Tanh)
                    t1 = sp.tile([128, RB, WC], f32, tag="t1F")
                    tt(t1[:], qF[:], hm2F[:].bitcast(f32), ALU.subtract)
                    t2 = sp.tile([128, RB, WC], f32, tag="t2F")
                    tt(t2[:], zF[:], t1[:], ALU.mult)
                    oF = sp.tile([128, RB, WC], f32, tag="oF")
                    tt(oF[:], hm2F[:].bitcast(f32), t2[:], ALU.add)
                    st(out_d[:, r0:r0 + RB, :], oF[:], dma)

    nc.finalize()
    return nc


def _pack_weights(w):
    """w [128, 448, kh, kw] with one of kh/kw == 1 -> [128, 4, 5, 128].

    ci tiles: t0 = h channels 0:128; t1..t3 = x channels at offsets
    XOFF with sizes 112/112/96 (all tile_size 128 on the PE)."""
    wk = np.asarray(w, np.float32).reshape(128, 448, 5)
    out = np.zeros((128, 4, 5, 128), np.float32)
    out[:, 0] = np.transpose(wk[:, 0:128, :], (1, 2, 0))
    for t in range(1, 4):
        o = 128 + XOFF[t - 1]
        ksz = KS[t]
        out[:ksz, t] = np.transpose(wk[:, o:o + ksz, :], (1, 2, 0))
    return out


def kernel(h, x1, x2, wz1, bz1, wr1, br1, wq1, bq1, wz2, bz2, wr2, br2,
           wq2, bq2):
    import os
    from concourse.bass_utils import run_bass_kernel_spmd

    if "nc" not in _CACHE:
        _CACHE["nc"] = _build_nc()
    nc = _CACHE["nc"]

    shared = {
        "w_z1": _pack_weights(wz1), "w_r1": _pack_weights(wr1),
        "w_q1": _pack_weights(wq1), "w_z2": _pack_weights(wz2),
        "w_r2": _pack_weights(wr2), "w_q2": _pack_weights(wq2),
        "ball": np.stack([np.asarray(b, np.float32) for b in
                          (bz1, br1, bq1, bz2, br2, bq2)], axis=1),
        "ident": np.eye(128, dtype=np.float32),
        "zeros": np.zeros((128, 128), np.float32),
    }
    h = np.asarray(h, np.float32)
    x1 = np.asarray(x1, np.float32)
    x2 = np.asarray(x2, np.float32)

    in_maps = []
    for c in range(8):
        b, s = divmod(c, 2)
        cols = slice(0, WC) if s == 0 else slice(W - WC, W)
        in_maps.append(dict(
            shared,
            h=np.ascontiguousarray(h[b][:, :, cols]),
            x1=np.ascontiguousarray(x1[b][:, :, cols]),
            x2=np.ascontiguousarray(x2[b][:, :, cols]),
        ))

    kr = run_bass_kernel_spmd(nc, in_maps, list(range(8)),
                              trace=bool(os.environ.get("KTRACE")))
    _CACHE["exec_time_ns"] = kr.exec_time_ns
    res = kr.results

    out = np.empty((B, HID, H, W), np.float32)
    for c in range(8):
        b, s = divmod(c, 2)
        o = res[c]["out"]
        if s == 0:
            out[b][:, :, 0:80] = o[:, :, 0:80]
        else:
            out[b][:, :, 80:160] = o[:, :, WC - 80:WC]
    return out


# revision 25
# speedup vs baseline: 1.2018x; 1.2018x over previous
"""DeltaSepConvGRU Trainium2 kernel.

Sharding: 8 cores = batch(4) x width-split(2). Each core gets an 88-column
window (80 output cols + 8-col halo on the interior side) of one batch image
and computes the full two-step GRU locally; no collectives. Horizontal convs
need the halo (4 chained (1,5) convs -> 8 cols); vertical (5,1) convs run on
the full 64-row height. Conv zero-padding is realized by clamping taps to
valid input ranges (skipped taps contribute zero, matching the reference's
zero padding); clamped matmuls write row/column-offset PSUM subranges, with a
full-coverage matmul always first in each accumulation group.

All conv matmuls run in float32r (fp32 with 11-bit mantissa, full PE speed);
elementwise math is fp32. Stage intermediates stream through packed DRAM
scratch tensors (one big contiguous DMA instead of several small ones) and
DMA dispatch is split between the SP and Pool sequencers.
"""
import sys
sys.path.insert(0, '/opt/trn_rl_repo')
import numpy as np

B, HID, INP = 4, 128, 320
H, W = 64, 160
WC = 88           # per-core window width
WP = WC + 4       # padded width for horizontal-conv inputs
RB = 8            # rows per block
NB = H // RB
NG = 2            # psum groups per block
GR = 4            # rows per group
N = GR * WC       # matmul free size (352)
THR = 0.02
KS = [128, 112, 112, 96]    # ci tiles: t0 = h, t1..t3 = x (all round to tile_size 128)
XOFF = [0, 112, 224]        # x-channel offsets of tiles t1..t3
WNAMES = ["z1", "r1", "q1", "z2", "r2", "q2"]
TAPS = [2, 0, 1, 3, 4]      # full-coverage tap first (s=0)

_CACHE = {}
REPS = 1
MODE = "full"  # full | pe | ew | nodma


def _build_nc():
    import concourse.bass as bass
    from concourse import bacc, mybir
    import concourse.tile as tile

    f32 = mybir.dt.float32
    f32r = mybir.dt.float32r
    AF = mybir.ActivationFunctionType
    ALU = mybir.AluOpType

    nc = bacc.Bacc(None, target_bir_lowering=False, debug=False)

    h_d = nc.dram_tensor("h", [128, H, WC], f32r, kind="ExternalInput")
    x1_d = nc.dram_tensor("x1", [INP, H, WC], f32r, kind="ExternalInput")
    x2_d = nc.dram_tensor("x2", [INP, H, WC], f32r, kind="ExternalInput")
    w_d = {g: nc.dram_tensor(f"w_{g}", [128, 4, 5, 128], f32, kind="ExternalInput")
           for g in WNAMES}
    ball_d = nc.dram_tensor("ball", [128, 6], f32, kind="ExternalInput")
    ident_d = nc.dram_tensor("ident", [128, 128], f32, kind="ExternalInput")
    zeros_d = nc.dram_tensor("zeros", [128, 128], f32r, kind="ExternalInput")
    out_d = nc.dram_tensor("out", [128, H, WC], f32, kind="ExternalOutput")

    # Packed DRAM scratch
    S_czrq1 = nc.dram_tensor("S_czrq1", [128, 3, H, WC], f32r)   # cz1,cr1,cq1
    S_rh1 = nc.dram_tensor("S_rh1", [128, 2, H, WC], f32r)       # r1h, hmid1
    S_bz = nc.dram_tensor("S_bz", [128, 4, H, WC], f32r)         # cz2,cr2,xq2,r2h
    S_cqh1 = nc.dram_tensor("S_cqh1", [128, 2, H, WC], f32r)     # cq2, h1
    S_dx = nc.dram_tensor("S_dx", [128, 3, H, WC], f32r)
    S_hm2 = nc.dram_tensor("S_hm2", [128, H, WC], f32r)
    S_de = nc.dram_tensor("S_de", [128, 2, H, WC], f32r)         # dhmid, dr2h

    def dma(out, in_):
        nc.sync.dma_start(out=out, in_=in_)

    def dmap(out, in_):
        nc.gpsimd.dma_start(out=out, in_=in_)

    def st(out, in_, via):
        if MODE == "pe":
            return
        via(out, in_)

    def zin(p, rows):
        return bass.AP(tensor=zeros_d[:].tensor, offset=0,
                       ap=[[128, p], [0, rows], [1, 2]])

    def zero_borders(tile4, rb, nt=None):
        """Zero the 2-col borders of a WP-padded tile; pool slots keep the
        zeros across reuse, so only the first `bufs` blocks need it."""
        if rb >= 3:
            return
        if nt is None:
            dmap(tile4[:, :, 0:2], zin(128, tile4.shape[1]))
            dmap(tile4[:, :, 90:92], zin(128, tile4.shape[1]))
        else:
            for t in range(nt):
                dmap(tile4[:, t, :, 0:2], zin(128, tile4.shape[2]))
                dmap(tile4[:, t, :, 90:92], zin(128, tile4.shape[2]))

    def slot_ap(src, nslots, slot0, step, count, r0, rows):
        """AP over packed scratch [128, nslots, H, WC]: slots
        slot0, slot0+step, ...; rows [r0, r0+rows)."""
        hw_ = H * WC
        return bass.AP(tensor=src[:].tensor, offset=slot0 * hw_ + r0 * WC,
                       ap=[[nslots * hw_, 128], [step * hw_, count],
                           [WC, rows], [1, WC]])

    def x_load(tile_, src, r0, rows, via=None, per_slot=False):
        """Load x-like [320,H,WC] rows into packed [128,3,rows,WC] tile view."""
        d = via or dma
        hw_ = H * WC
        if per_slot:
            for t in range(3):
                d(tile_[0:KS[t + 1], t, :, :],
                  bass.AP(tensor=src[:].tensor,
                          offset=XOFF[t] * hw_ + r0 * WC,
                          ap=[[hw_, KS[t + 1]], [WC, rows], [1, WC]]))
            return
        d(tile_[0:112, 0:2, :, :],
          bass.AP(tensor=src[:].tensor, offset=r0 * WC,
                  ap=[[hw_, 112], [112 * hw_, 2], [WC, rows], [1, WC]]))
        d(tile_[0:96, 2, :, :],
          bass.AP(tensor=src[:].tensor, offset=224 * hw_ + r0 * WC,
                  ap=[[hw_, 96], [WC, rows], [1, WC]]))

    def dx_load(tile_, r0, rows, via=None, per_slot=False):
        d = via or dma
        hw_ = H * WC
        if per_slot:
            for t in range(3):
                d(tile_[0:KS[t + 1], t, :, :],
                  bass.AP(tensor=S_dx[:].tensor, offset=t * hw_ + r0 * WC,
                          ap=[[3 * hw_, KS[t + 1]], [WC, rows], [1, WC]]))
            return
        d(tile_[0:112, 0:2, :, :],
          bass.AP(tensor=S_dx[:].tensor, offset=r0 * WC,
                  ap=[[3 * hw_, 112], [hw_, 2], [WC, rows], [1, WC]]))
        d(tile_[0:96, 2, :, :],
          bass.AP(tensor=S_dx[:].tensor, offset=2 * hw_ + r0 * WC,
                  ap=[[3 * hw_, 96], [WC, rows], [1, WC]]))

    def mm_run(psum3, mms):
        if MODE == "ew":
            return
        for i, (lh, rh, orows, ocols) in enumerate(mms):
            nc.tensor.matmul(psum3[:, orows, ocols], lhsT=lh, rhs=rh,
                             start=(i == 0), stop=(i == len(mms) - 1))

    def mm_horiz(psumc, w, h3, x4, g, extra=None):
        # psumc: cols-major view [128, WC, GR]; column subranges contiguous
        rows = slice(g * GR, (g + 1) * GR)

        def cr(ap):
            return ap.rearrange("p r c -> p c r")

        mms = []
        if extra is not None:
            mms.append((extra[0], cr(extra[1]), slice(0, WC), slice(0, GR)))
        for k in TAPS:
            s = k - 2
            a, b = max(0, -s), min(WC, WC - s)
            oc = slice(a, b)
            mms.append((w[:, 0, k, :], cr(h3[:, rows, a + s:b + s]),
                        oc, slice(0, GR)))
            for t in (1, 2, 3):
                mms.append((w[0:KS[t], t, k, :],
                            cr(x4[0:KS[t], t - 1, rows, a + s:b + s]),
                            oc, slice(0, GR)))
        mm_run(psumc, mms)

    def mm_vert(psum3, w, vh3, vx4, g, rb, extra=None, x_only=False,
                h_only=False):
        r0 = rb * RB
        q0 = r0 + g * GR
        base = r0 - 2      # global row of local row 0 in the 12-row tile
        mms = []
        if extra is not None:
            mms.append((extra[0], extra[1], slice(0, GR), slice(0, WC)))
        for k in TAPS:
            s = k - 2
            a_r = max(q0, -s)
            b_r = min(q0 + GR, H - s)
            orows = slice(a_r - q0, b_r - q0)
            la, lb = a_r + s - base, b_r + s - base
            if not x_only:
                mms.append((w[:, 0, k, :], vh3[:, la:lb, :], orows,
                            slice(0, WC)))
            if not h_only:
                for t in (1, 2, 3):
                    mms.append((w[0:KS[t], t, k, :],
                                vx4[0:KS[t], t - 1, la:lb, :], orows,
                                slice(0, WC)))
        mm_run(psum3, mms)

    def p3(psum):
        return psum[:].rearrange("p (r w) -> p r w", w=WC)

    def p3c(psum):
        return psum[:].rearrange("p (w r) -> p w r", w=WC)

    def crv(ap):
        return ap.rearrange("p r c -> p c r")

    def act(out, in_, func, bias=0.0):
        if MODE == "pe":
            return
        nc.scalar.activation(out=out, in_=in_, func=func, bias=bias, scale=1.0)

    def tt(out, in0, in1, op):
        if MODE == "pe":
            return
        nc.vector.tensor_tensor(out=out, in0=in0, in1=in1, op=op)

    def delta_chain(t, sq, d, new, old):
        if MODE == "pe":
            return
        tt(t, new, old, ALU.subtract)
        tt(sq, t, t, ALU.mult)
        nc.vector.scalar_tensor_tensor(out=d, in0=sq, scalar=THR * THR,
                                       in1=t, op0=ALU.is_gt, op1=ALU.mult)

    def halo_rows(rb):
        """(local row range loaded, global row range) of a 12-row halo tile."""
        r0 = rb * RB
        lo, hi = max(r0 - 2, 0), min(r0 + RB + 2, H)
        return lo - (r0 - 2), hi - (r0 - 2), lo, hi

    with tile.TileContext(nc) as tc:
        with tc.tile_pool(name="glob", bufs=1) as gp:
            ident_stg = gp.tile([128, 128], f32)
            dma(ident_stg[:], ident_d[:])
            ident_r = gp.tile([128, 128], f32r)
            nc.vector.tensor_copy(out=ident_r[:], in_=ident_stg[:])
            ball = gp.tile([128, 6], f32)
            dma(ball[:], ball_d[:])
            bias = {g: ball[:, i:i + 1] for i, g in enumerate(WNAMES)}

            wsall = {}
            with tc.tile_pool(name="wstgp", bufs=2) as wsp:
                for g in WNAMES:
                    stg = wsp.tile([128, 4, 5, 128], f32, tag="wstg",
                                   name=f"wstg_{g}")
                    dma(stg[:], w_d[g][:])
                    wsall[g] = gp.tile([128, 4, 5, 128], f32r, tag=f"w_{g}",
                                       name=f"w_{g}")
                    nc.vector.tensor_copy(out=wsall[g][:], in_=stg[:])

            for _rep in range(REPS):
              # ------------- Stage X: d_x = delta(x2 - x1) -> S_dx ----------
              with tc.tile_pool(name="stX", bufs=2) as sp:
                for rb in range(NB):
                    r0 = rb * RB
                    x1X = sp.tile([128, 3, RB, WC], f32r, tag="x1X", bufs=3)
                    x2X = sp.tile([128, 3, RB, WC], f32r, tag="x2X", bufs=3)
                    x_load(x1X, x1_d, r0, RB)
                    x_load(x2X, x2_d, r0, RB)
                    tX = sp.tile([128, 3, RB, WC], f32, tag="tX")
                    sqX = sp.tile([128, 3, RB, WC], f32, tag="sqX")
                    dxX = sp.tile([128, 3, RB, WC], f32r, tag="dxX")
                    for pr, ts in ((slice(0, 112), slice(0, 2)),
                                   (slice(0, 96), slice(2, 3))):
                        delta_chain(tX[pr, ts], sqX[pr, ts], dxX[pr, ts],
                                    x2X[pr, ts].bitcast(f32),
                                    x1X[pr, ts].bitcast(f32))
                    st(S_dx[0:112, 0:2, r0:r0 + RB, :], dxX[0:112, 0:2], dmap)
                    st(S_dx[0:96, 2, r0:r0 + RB, :], dxX[0:96, 2], dmap)

              # ------------- Stage A: step1 horizontal ----------------------
              with tc.tile_pool(name="stA", bufs=2) as sp, \
                   tc.tile_pool(name="psA", bufs=2, space="PSUM") as pp:
                ws = wsall
                for rb in range(NB):
                    r0 = rb * RB
                    hA = sp.tile([128, RB, WC], f32r, tag="hA", bufs=3)
                    dma(hA[:], h_d[:, r0:r0 + RB, :])
                    xA = sp.tile([128, 3, RB, WC], f32r, tag="xA", bufs=3)
                    x_load(xA, x1_d, r0, RB)
                    czrqA = sp.tile([128, 3, RB, WC], f32r, tag="czrqA")
                    rhA = sp.tile([128, 2, RB, WC], f32r, tag="rhA")
                    zA = sp.tile([128, RB, WC], f32, tag="zA")
                    rA = sp.tile([128, RB, WC], f32, tag="rA")
                    qA = sp.tile([128, RB, WC], f32, tag="qA")
                    for g in range(NG):
                        rows = slice(g * GR, (g + 1) * GR)
                        pz = pp.tile([128, N], f32, tag="pz")
                        mm_horiz(p3c(pz), ws["z1"], hA, xA, g)
                        act(crv(czrqA[:, 0, rows, :]), p3c(pz), AF.Identity,
                            bias["z1"])
                        act(crv(zA[:, rows, :]), p3c(pz), AF.Sigmoid,
                            bias["z1"])
                        pr = pp.tile([128, N], f32, tag="pr")
                        mm_horiz(p3c(pr), ws["r1"], hA, xA, g)
                        act(crv(czrqA[:, 1, rows, :]), p3c(pr), AF.Identity,
                            bias["r1"])
                        act(crv(rA[:, rows, :]), p3c(pr), AF.Sigmoid,
                            bias["r1"])
                        tt(rhA[:, 0, rows, :], rA[:, rows, :],
                           hA[:, rows, :].bitcast(f32), ALU.mult)
                        pq = pp.tile([128, N], f32, tag="pq")
                        mm_horiz(p3c(pq), ws["q1"], rhA[:, 0], xA, g)
                        act(crv(czrqA[:, 2, rows, :]), p3c(pq), AF.Identity,
                            bias["q1"])
                        act(crv(qA[:, rows, :]), p3c(pq), AF.Tanh,
                            bias["q1"])
                    t1 = sp.tile([128, RB, WC], f32, tag="t1A")
                    tt(t1[:], qA[:], hA[:].bitcast(f32), ALU.subtract)
                    t2 = sp.tile([128, RB, WC], f32, tag="t2A")
                    tt(t2[:], zA[:], t1[:], ALU.mult)
                    tt(rhA[:, 1], hA[:].bitcast(f32), t2[:], ALU.add)
                    st(S_czrq1[:, :, r0:r0 + RB, :], czrqA[:], dmap)
                    st(S_rh1[:, :, r0:r0 + RB, :], rhA[:], dmap)

              # ------------- Stage B: step1 vertical (cz2,cr2,xq2,r2h) ------
              with tc.tile_pool(name="stB", bufs=2) as sp, \
                   tc.tile_pool(name="psB", bufs=2, space="PSUM") as pp:
                ws = wsall
                for rb in range(NB):
                    r0 = rb * RB
                    la, lb, glo, ghi = halo_rows(rb)
                    vh = sp.tile([128, RB + 4, WC], f32r, tag="vhB", bufs=3)
                    dma(vh[:, la:lb, :], slot_ap(S_rh1, 2, 1, 1, 1, glo,
                                                 ghi - glo))
                    vx = sp.tile([128, 3, RB + 4, WC], f32r, tag="vxB", bufs=3)
                    x_load(vx[:, :, la:lb, :], x1_d, glo, ghi - glo)
                    bzB = sp.tile([128, 4, RB, WC], f32r, tag="bzB")
                    rB = sp.tile([128, RB, WC], f32, tag="rB")
                    for g in range(NG):
                        rows = slice(g * GR, (g + 1) * GR)
                        pz = pp.tile([128, N], f32, tag="pzB")
                        mm_vert(p3(pz), ws["z2"], vh, vx, g, rb)
                        act(bzB[:, 0, rows, :], p3(pz), AF.Identity,
                            bias["z2"])
                        pr = pp.tile([128, N], f32, tag="prB")
                        mm_vert(p3(pr), ws["r2"], vh, vx, g, rb)
                        act(bzB[:, 1, rows, :], p3(pr), AF.Identity,
                            bias["r2"])
                        act(rB[:, rows, :], p3(pr), AF.Sigmoid, bias["r2"])
                        px = pp.tile([128, N], f32, tag="pxB")
                        mm_vert(p3(px), ws["q2"], vh, vx, g, rb, x_only=True)
                        act(bzB[:, 2, rows, :], p3(px), AF.Identity,
                            bias["q2"])
                    tt(bzB[:, 3], rB[:], vh[:, 2:RB + 2, :].bitcast(f32),
                       ALU.mult)
                    st(S_bz[:, :, r0:r0 + RB, :], bzB[:], dmap)

              # ------------- Stage C: step1 vertical (cq2, h1) --------------
              with tc.tile_pool(name="stC", bufs=2) as sp, \
                   tc.tile_pool(name="psC", bufs=2, space="PSUM") as pp:
                ws = wsall
                for rb in range(NB):
                    r0 = rb * RB
                    la, lb, glo, ghi = halo_rows(rb)
                    vr = sp.tile([128, RB + 4, WC], f32r, tag="vrC", bufs=3)
                    dma(vr[:, la:lb, :], slot_ap(S_bz, 4, 3, 1, 1, glo,
                                                 ghi - glo))
                    czxC = sp.tile([128, 2, RB, WC], f32r, tag="czxC", bufs=3)
                    dma(czxC[:], slot_ap(S_bz, 4, 0, 2, 2, r0, RB))
                    hm1C = sp.tile([128, RB, WC], f32r, tag="hm1C", bufs=3)
                    dma(hm1C[:], slot_ap(S_rh1, 2, 1, 1, 1, r0, RB))
                    chC = sp.tile([128, 2, RB, WC], f32r, tag="chC")
                    qC = sp.tile([128, RB, WC], f32, tag="qC")
                    for g in range(NG):
                        rows = slice(g * GR, (g + 1) * GR)
                        pq = pp.tile([128, N], f32, tag="pqC")
                        mm_vert(p3(pq), ws["q2"], vr, None, g, rb, h_only=True,
                                extra=(ident_r[:], czxC[:, 1, rows, :]))
                        act(chC[:, 0, rows, :], p3(pq), AF.Identity)
                        act(qC[:, rows, :], p3(pq), AF.Tanh)
                    zC = sp.tile([128, RB, WC], f32, tag="zC")
                    act(zC[:], czxC[:, 0].bitcast(f32), AF.Sigmoid)
                    t1 = sp.tile([128, RB, WC], f32, tag="t1C")
                    tt(t1[:], qC[:], hm1C[:].bitcast(f32), ALU.subtract)
                    t2 = sp.tile([128, RB, WC], f32, tag="t2C")
                    tt(t2[:], zC[:], t1[:], ALU.mult)
                    tt(chC[:, 1], hm1C[:].bitcast(f32), t2[:], ALU.add)
                    st(S_cqh1[:, :, r0:r0 + RB, :], chC[:], dmap)

              # ------------- Stage D: step2 horizontal ----------------------
              with tc.tile_pool(name="stD", bufs=2) as sp, \
                   tc.tile_pool(name="psD", bufs=2, space="PSUM") as pp:
                ws = wsall
                for rb in range(NB):
                    r0 = rb * RB
                    hD = sp.tile([128, RB, WC], f32r, tag="hD", bufs=3)
                    dmap(hD[:], h_d[:, r0:r0 + RB, :])
                    h1D = sp.tile([128, RB, WC], f32r, tag="h1D", bufs=3)
                    dmap(h1D[:], slot_ap(S_cqh1, 2, 1, 1, 1, r0, RB))
                    dx = sp.tile([128, 3, RB, WC], f32r, tag="dxD", bufs=3)
                    dx_load(dx, r0, RB, via=dmap)
                    r1hD = sp.tile([128, RB, WC], f32r, tag="r1hD", bufs=3)
                    dmap(r1hD[:], slot_ap(S_rh1, 2, 0, 1, 1, r0, RB))
                    czrqD = sp.tile([128, 3, RB, WC], f32r, tag="czrqD", bufs=3)
                    dmap(czrqD[:], S_czrq1[:, :, r0:r0 + RB, :])

                    tD = sp.tile([128, RB, WC], f32, tag="tD")
                    sqD = sp.tile([128, RB, WC], f32, tag="sqD")
                    dh = sp.tile([128, RB, WC], f32r, tag="dh")
                    delta_chain(tD[:], sqD[:], dh[:], h1D[:].bitcast(f32),
                                hD[:].bitcast(f32))
                    dr1h = sp.tile([128, RB, WC], f32r, tag="dr1h")
                    zD = sp.tile([128, RB, WC], f32, tag="zD")
                    qD = sp.tile([128, RB, WC], f32, tag="qD")
                    for g in range(NG):
                        rows = slice(g * GR, (g + 1) * GR)
                        pz = pp.tile([128, N], f32, tag="pzD")
                        mm_horiz(p3c(pz), ws["z1"], dh, dx, g,
                                 extra=(ident_r[:], czrqD[:, 0, rows, :]))
                        act(crv(zD[:, rows, :]), p3c(pz), AF.Sigmoid)
                        prr = pp.tile([128, N], f32, tag="prD")
                        mm_horiz(p3c(prr), ws["r1"], dh, dx, g,
                                 extra=(ident_r[:], czrqD[:, 1, rows, :]))
                        rD = sp.tile([128, GR, WC], f32, tag="rD")
                        act(crv(rD[:]), p3c(prr), AF.Sigmoid)
                        r1n = sp.tile([128, GR, WC], f32, tag="r1n")
                        tt(r1n[:], rD[:], h1D[:, rows, :].bitcast(f32),
                           ALU.mult)
                        tg = sp.tile([128, GR, WC], f32, tag="tg")
                        sqg = sp.tile([128, GR, WC], f32, tag="sqg")
                        delta_chain(tg[:], sqg[:], dr1h[:, rows, :], r1n[:],
                                    r1hD[:, rows, :].bitcast(f32))
                        pq = pp.tile([128, N], f32, tag="pqD")
                        mm_horiz(p3c(pq), ws["q1"], dr1h, dx, g,
                                 extra=(ident_r[:], czrqD[:, 2, rows, :]))
                        act(crv(qD[:, rows, :]), p3c(pq), AF.Tanh)
                    t1 = sp.tile([128, RB, WC], f32, tag="t1D")
                    tt(t1[:], qD[:], h1D[:].bitcast(f32), ALU.subtract)
                    t2 = sp.tile([128, RB, WC], f32, tag="t2D")
                    tt(t2[:], zD[:], t1[:], ALU.mult)
                    hm2D = sp.tile([128, RB, WC], f32r, tag="hm2D")
                    tt(hm2D[:], h1D[:].bitcast(f32), t2[:], ALU.add)
                    st(S_hm2[:, r0:r0 + RB, :], hm2D[:], dma)

              # ------------- Stage E: step2 vertical pass A -----------------
              with tc.tile_pool(name="stE", bufs=2) as sp, \
                   tc.tile_pool(name="psE", bufs=2, space="PSUM") as pp:
                ws = wsall
                for rb in range(NB):
                    r0 = rb * RB
                    la, lb, glo, ghi = halo_rows(rb)
                    hm2E = sp.tile([128, RB + 4, WC], f32r, tag="hm2E", bufs=3)
                    dmap(hm2E[:, la:lb, :], S_hm2[:, glo:ghi, :])
                    hm1E = sp.tile([128, RB + 4, WC], f32r, tag="hm1E", bufs=3)
                    dmap(hm1E[:, la:lb, :], slot_ap(S_rh1, 2, 1, 1, 1, glo,
                                                   ghi - glo))
                    dxE = sp.tile([128, 3, RB + 4, WC], f32r, tag="dxE", bufs=3)
                    dx_load(dxE[:, :, la:lb, :], glo, ghi - glo, via=dmap)
                    crrE = sp.tile([128, 2, RB, WC], f32r, tag="crrE", bufs=3)
                    dmap(crrE[:], slot_ap(S_bz, 4, 1, 2, 2, r0, RB))

                    tE = sp.tile([128, RB + 4, WC], f32, tag="tE")
                    sqE = sp.tile([128, RB + 4, WC], f32, tag="sqE")
                    dhm = sp.tile([128, RB + 4, WC], f32r, tag="dhm")
                    delta_chain(tE[:, la:lb, :], sqE[:, la:lb, :],
                                dhm[:, la:lb, :],
                                hm2E[:, la:lb, :].bitcast(f32),
                                hm1E[:, la:lb, :].bitcast(f32))
                    rE = sp.tile([128, RB, WC], f32, tag="rE")
                    for g in range(NG):
                        rows = slice(g * GR, (g + 1) * GR)
                        pr = pp.tile([128, N], f32, tag="prE")
                        mm_vert(p3(pr), ws["r2"], hm2E if MODE == "pe" else dhm, dxE, g, rb,
                                extra=(ident_r[:], crrE[:, 0, rows, :]))
                        act(rE[:, rows, :], p3(pr), AF.Sigmoid)
                    deE = sp.tile([128, 2, RB, WC], f32r, tag="deE")
                    r2n = sp.tile([128, RB, WC], f32, tag="r2n")
                    tt(r2n[:], rE[:], hm2E[:, 2:RB + 2, :].bitcast(f32),
                       ALU.mult)
                    tE2 = sp.tile([128, RB, WC], f32, tag="tE2")
                    sqE2 = sp.tile([128, RB, WC], f32, tag="sqE2")
                    delta_chain(tE2[:], sqE2[:], deE[:, 1], r2n[:],
                                crrE[:, 1].bitcast(f32))
                    if MODE != "pe":
                        nc.vector.tensor_copy(out=deE[:, 0],
                                              in_=dhm[:, 2:RB + 2, :])
                    st(S_de[:, :, r0:r0 + RB, :], deE[:], dma)

              # ------------- Stage F: step2 vertical pass B -----------------
              with tc.tile_pool(name="stF", bufs=2) as sp, \
                   tc.tile_pool(name="psF", bufs=2, space="PSUM") as pp:
                ws = wsall
                for rb in range(NB):
                    r0 = rb * RB
                    la, lb, glo, ghi = halo_rows(rb)
                    deF = sp.tile([128, 2, RB + 4, WC], f32r, tag="deF", bufs=3)
                    dmap(deF[:, :, la:lb, :],
                        bass.AP(tensor=S_de[:].tensor, offset=glo * WC,
                                ap=[[2 * H * WC, 128], [H * WC, 2],
                                    [WC, ghi - glo], [1, WC]]))
                    dxF = sp.tile([128, 3, RB + 4, WC], f32r, tag="dxF", bufs=3)
                    dx_load(dxF[:, :, la:lb, :], glo, ghi - glo, via=dmap)
                    czF = sp.tile([128, RB, WC], f32r, tag="czF", bufs=3)
                    dmap(czF[:], slot_ap(S_bz, 4, 0, 1, 1, r0, RB))
                    cqF = sp.tile([128, RB, WC], f32r, tag="cqF", bufs=3)
                    dmap(cqF[:], slot_ap(S_cqh1, 2, 0, 1, 1, r0, RB))
                    hm2F = sp.tile([128, RB, WC], f32r, tag="hm2F", bufs=3)
                    dmap(hm2F[:], S_hm2[:, r0:r0 + RB, :])
                    zF = sp.tile([128, RB, WC], f32, tag="zF")
                    qF = sp.tile([128, RB, WC], f32, tag="qF")
                    for g in range(NG):
                        rows = slice(g * GR, (g + 1) * GR)
                        pz = pp.tile([128, N], f32, tag="pzF")
                        mm_vert(p3(pz), ws["z2"], deF[:, 0], dxF, g, rb,
                                extra=(ident_r[:], czF[:, rows, :]))
                        act(zF[:, rows, :], p3(pz), AF.Sigmoid)
                        pq = pp.tile([128, N], f32, tag="pqF")
                        mm_vert(p3(pq), ws["q2"], deF[:, 1], dxF, g, rb,
                                extra=(ident_r[:], cqF[:, rows, :]))
                        act(qF[:, rows, :], p3(pq), AF.Tanh)
                    t1 = sp.tile([128, RB, WC], f32, tag="t1F")
                    tt(t1[:], qF[:], hm2F[:].bitcast(f32), ALU.subtract)
                    t2 = sp.tile([128, RB, WC], f32, tag="t2F")
                    tt(t2[:], zF[:], t1[:], ALU.mult)
                    oF = sp.tile([128, RB, WC], f32, tag="oF")
                    tt(oF[:], hm2F[:].bitcast(f32), t2[:], ALU.add)
                    st(out_d[:, r0:r0 + RB, :], oF[:], dma)

    nc.finalize()
    return nc


def _pack_weights(w):
    """w [128, 448, kh, kw] with one of kh/kw == 1 -> [128, 4, 5, 128].

    ci tiles: t0 = h channels 0:128; t1..t3 = x channels at offsets
    XOFF with sizes 112/112/96 (all tile_size 128 on the PE)."""
    wk = np.asarray(w, np.float32).reshape(128, 448, 5)
    out = np.zeros((128, 4, 5, 128), np.float32)
    out[:, 0] = np.transpose(wk[:, 0:128, :], (1, 2, 0))
    for t in range(1, 4):
        o = 128 + XOFF[t - 1]
        ksz = KS[t]
        out[:ksz, t] = np.transpose(wk[:, o:o + ksz, :], (1, 2, 0))
    return out


def kernel(h, x1, x2, wz1, bz1, wr1, br1, wq1, bq1, wz2, bz2, wr2, br2,
           wq2, bq2):
    import os
    from concourse.bass_utils import run_bass_kernel_spmd

    if "nc" not in _CACHE:
        _CACHE["nc"] = _build_nc()
    nc = _CACHE["nc"]

    shared = {
        "w_z1": _pack_weights(wz1), "w_r1": _pack_weights(wr1),
        "w_q1": _pack_weights(wq1), "w_z2": _pack_weights(wz2),
        "w_r2": _pack_weights(wr2), "w_q2": _pack_weights(wq2),
        "ball": np.stack([np.asarray(b, np.float32) for b in
                          (bz1, br1, bq1, bz2, br2, bq2)], axis=1),
        "ident": np.eye(128, dtype=np.float32),
        "zeros": np.zeros((128, 128), np.float32),
    }
    h = np.asarray(h, np.float32)
    x1 = np.asarray(x1, np.float32)
    x2 = np.asarray(x2, np.float32)

    in_maps = []
    for c in range(8):
        b, s = divmod(c, 2)
        cols = slice(0, WC) if s == 0 else slice(W - WC, W)
        in_maps.append(dict(
            shared,
            h=np.ascontiguousarray(h[b][:, :, cols]),
            x1=np.ascontiguousarray(x1[b][:, :, cols]),
            x2=np.ascontiguousarray(x2[b][:, :, cols]),
        ))

    kr = run_bass_kernel_spmd(nc, in_maps, list(range(8)),
                              trace=bool(os.environ.get("KTRACE")))
    _CACHE["exec_time_ns"] = kr.exec_time_ns
    res = kr.results

    out = np.empty((B, HID, H, W), np.float32)
    for c in range(8):
        b, s = divmod(c, 2)
        o = res[c]["out"]
        if s == 0:
            out[b][:, :, 0:80] = o[:, :, 0:80]
        else:
            out[b][:, :, 80:160] = o[:, :, WC - 80:WC]
    return out
